# revision 9
# baseline (speedup 1.0000x reference)
"""Trainium2 Bass kernel for the 4-layer Mamba-style GBM model.

Sharding: 8 cores = 4 batches x 2 d_inner halves. Each core handles one
batch and one 512-channel half of d_inner; the two cores of a batch pair
all-reduce the xproj output (dbl) and the out_proj partial sums.

Layout: activations are feature-major in SBUF ([d on partitions, t on
free]).  The selective scan runs as 64 native tensor_tensor_scan calls
per layer (4 d-groups x 16 state dims) with decays dA_n = exp(n*ln r)
produced on the scalar engine from lnr = Ln(sigmoid(-dt_raw)), using
softplus(a) = -ln(sigmoid(-a)).
"""
import sys
sys.path.insert(0, "/opt/trn_rl_repo")

import numpy as np
import ml_dtypes

import concourse.bacc as bacc
import concourse.tile as tile
from concourse import mybir
from concourse.bass_utils import run_bass_kernel_spmd

F32 = mybir.dt.float32
BF16 = mybir.dt.bfloat16
AF = mybir.ActivationFunctionType
OP = mybir.AluOpType
AX = mybir.AxisListType

D_MODEL = 512
D_LOC = 512          # d_inner half per core
N = 16               # d_state
S = 1024
KCONV = 4
NLAYERS = 4
LATENT = 1024
BATCH = 4
GROUPS = [[0, 1], [2, 3], [4, 5], [6, 7]]
NV = 96              # pvec columns

_CACHE = {}


def _body(nc, tc, dram, out_d):
    import contextlib
    ctx = contextlib.ExitStack()
    with ctx:
        persist = ctx.enter_context(tc.tile_pool(name="persist", bufs=1))
        wbig = ctx.enter_context(tc.tile_pool(name="wbig", bufs=1))
        wsm = ctx.enter_context(tc.tile_pool(name="wsm", bufs=2))
        act = ctx.enter_context(tc.tile_pool(name="act", bufs=1))
        trans = ctx.enter_context(tc.tile_pool(name="trans", bufs=2))
        scanp = ctx.enter_context(tc.tile_pool(name="scanp", bufs=3))
        ps_mm = ctx.enter_context(tc.tile_pool(name="ps_mm", bufs=4, space="PSUM"))
        ps_sm = ctx.enter_context(tc.tile_pool(name="ps_sm", bufs=1, space="PSUM"))
        dpool = ctx.enter_context(tc.tile_pool(name="dpool", bufs=2, space="DRAM"))

        # ---- persistent small tensors
        pv = persist.tile([128, 4, NV], F32)
        nc.sync.dma_start(pv[:], dram["pvec"][:])
        l1b = persist.tile([128, 4], F32)
        nc.sync.dma_start(l1b[:], dram["lin1bT"][:])
        l2b = persist.tile([128, 8], F32)
        nc.sync.dma_start(l2b[:], dram["lin2bT"][:])
        ones_sb = persist.tile([128, 1], BF16)
        nc.sync.dma_start(ones_sb[:], dram["ones1"][:])
        ident_sb = persist.tile([128, 128], BF16)
        nc.sync.dma_start(ident_sb[:], dram["ident"][:])

        def pcol(g, c):
            return pv[:, g, c:c + 1]

        eps_t = persist.tile([1, 1], F32)
        nc.gpsimd.memset(eps_t[:], 1e-5)

        h = persist.tile([128, 4, S], F32)

        # ---- lin1: h = lin1w.T @ xT + b   (scoped pool, freed after)
        with tc.tile_pool(name="lin1p", bufs=1) as lp:
            xT_sb = lp.tile([128, 8, S], BF16)
            nc.sync.dma_start(xT_sb[:], dram["xT"][:])
            l1w = lp.tile([128, 8, 512], BF16)
            nc.sync.dma_start(l1w[:], dram["lin1w"][:])
            for m in range(4):
                for f in range(2):
                    ps = ps_mm.tile([128, 512], F32)
                    for kc in range(8):
                        nc.tensor.matmul(
                            ps[:], l1w[:, kc, m * 128:(m + 1) * 128],
                            xT_sb[:, kc, f * 512:(f + 1) * 512],
                            start=(kc == 0), stop=(kc == 7))
                    nc.scalar.activation(h[:, m, f * 512:(f + 1) * 512],
                                         ps[:], AF.Identity,
                                         bias=l1b[:, m:m + 1])

        # ---- layers (big scan tiles in a scoped pool, freed before tail)
        with tc.tile_pool(name="bigp", bufs=1) as big:
            for l in range(NLAYERS):
                inw_sb = wbig.tile([128, 4, 1024], BF16, tag="inw")
                nc.sync.dma_start(inw_sb[:], dram["inw"][l])
                outw_sb = wbig.tile([128, 4, 512], BF16, tag="outw")
                nc.sync.dma_start(outw_sb[:], dram["outw"][l])
                xprojw_sb = wsm.tile([128, 4, 64], BF16, tag="xprojw")
                nc.sync.dma_start(xprojw_sb[:], dram["xprojw"][l])
                dtw_sb = wsm.tile([32, 512], BF16, tag="dtw")
                nc.sync.dma_start(dtw_sb[:], dram["dtw"][l])

                # rmsnorm -> hn16
                sq = act.tile([128, 4, S], BF16, tag="sq")
                for g in range(4):
                    nc.scalar.activation(sq[:, g, :], h[:, g, :], AF.Square)
                s_t = trans.tile([1, S], F32, tag="s_t")
                for f in range(2):
                    pss = ps_sm.tile([1, 512], F32, tag="pnorm")
                    for kc in range(4):
                        nc.tensor.matmul(pss[:], ones_sb[:],
                                         sq[:, kc, f * 512:(f + 1) * 512],
                                         start=(kc == 0), stop=(kc == 3))
                    nc.scalar.activation(s_t[:, f * 512:(f + 1) * 512],
                                         pss[:], AF.Ln,
                                         scale=1.0 / D_MODEL, bias=eps_t[:])
                nc.scalar.activation(s_t[:], s_t[:], AF.Exp, scale=-0.5)
                s_dram = dpool.tile([1, S], F32, tag="s_dram")
                nc.sync.dma_start(s_dram[:], s_t[:])
                s_rep = trans.tile([128, S], F32, tag="s_rep")
                nc.sync.dma_start(s_rep[:], s_dram[:].broadcast_to([128, S]))
                hn16 = act.tile([128, 4, S], BF16, tag="hn16")
                for g in range(4):
                    nc.vector.scalar_tensor_tensor(
                        hn16[:, g, :], in0=h[:, g, :], scalar=pcol(g, l),
                        in1=s_rep[:], op0=OP.mult, op1=OP.mult)

                # in_proj -> xp_pad (pre-activation), sz16 = silu(z)
                xp_pad = act.tile([128, 4, S + 3], BF16, tag="xp_pad")
                nc.gpsimd.memset(xp_pad[:, :, 0:3], 0.0)
                sz16 = act.tile([128, 4, S], BF16, tag="sz16")
                for m in range(8):
                    for f in range(2):
                        ps = ps_mm.tile([128, 512], F32)
                        for kc in range(4):
                            nc.tensor.matmul(
                                ps[:], inw_sb[:, kc, m * 128:(m + 1) * 128],
                                hn16[:, kc, f * 512:(f + 1) * 512],
                                start=(kc == 0), stop=(kc == 3))
                        if m < 4:
                            nc.scalar.activation(
                                xp_pad[:, m, 3 + f * 512: 3 + (f + 1) * 512],
                                ps[:], AF.Copy)
                        else:
                            nc.scalar.activation(
                                sz16[:, m - 4, f * 512:(f + 1) * 512],
                                ps[:], AF.Silu)

                # causal depthwise conv + bias + silu -> xpa16
                xpa16 = act.tile([128, 4, S], BF16, tag="xpa16")
                for g in range(4):
                    c0 = trans.tile([128, S], BF16, tag="conv")
                    nc.vector.tensor_scalar_mul(c0[:], xp_pad[:, g, 0:S],
                                                pcol(g, 16 + 4 * l + 0))
                    for k in range(1, KCONV):
                        c1 = trans.tile([128, S], BF16, tag="conv")
                        nc.vector.scalar_tensor_tensor(
                            c1[:], in0=xp_pad[:, g, k:k + S],
                            scalar=pcol(g, 16 + 4 * l + k),
                            in1=c0[:], op0=OP.mult, op1=OP.add)
                        c0 = c1
                    nc.scalar.activation(xpa16[:, g, :], c0[:], AF.Silu,
                                         bias=pcol(g, 8 + l))

                # xproj -> dbl partial -> pair allreduce
                dblp = trans.tile([64, S], F32, tag="dblp")
                for f in range(2):
                    psx = ps_sm.tile([64, 512], F32, tag="pxproj")
                    for kc in range(4):
                        nc.tensor.matmul(psx[:], xprojw_sb[:, kc, :],
                                         xpa16[:, kc, f * 512:(f + 1) * 512],
                                         start=(kc == 0), stop=(kc == 3))
                    nc.scalar.activation(dblp[:, f * 512:(f + 1) * 512],
                                         psx[:], AF.Copy)
                dbl_in = dpool.tile([64, S], F32, tag="dbl_in")
                dbl_out = dpool.tile([64, S], F32, tag="dbl_out")
                nc.gpsimd.dma_start(dbl_in[:], dblp[:])
                nc.gpsimd.collective_compute(
                    "AllReduce", OP.add, replica_groups=GROUPS,
                    ins=[dbl_in[:].opt()], outs=[dbl_out[:].opt()])
                dbl16 = trans.tile([64, S], BF16, tag="dbl16")
                dbl_sb = trans.tile([64, S], F32, tag="dbl_sb")
                nc.gpsimd.dma_start(dbl_sb[:], dbl_out[:])
                nc.vector.tensor_copy(dbl16[:], dbl_sb[:])

                # broadcast B, C rows to all partitions via DRAM bounce
                bc_dram = dpool.tile([32, S], BF16, tag="bc_dram")
                nc.gpsimd.dma_start(bc_dram[:], dbl16[32:64, :])

                # dt-proj -> lnr = Ln(sigmoid(-(dt_raw + dt_b))) = -dt
                lnr16 = act.tile([128, 4, S], BF16, tag="xp_pad")
                for m in range(4):
                    for f in range(2):
                        ps = ps_mm.tile([128, 512], F32)
                        nc.tensor.matmul(
                            ps[:], dtw_sb[:, m * 128:(m + 1) * 128],
                            dbl16[0:32, f * 512:(f + 1) * 512],
                            start=True, stop=True)
                        nc.scalar.activation(
                            lnr16[:, m, f * 512:(f + 1) * 512], ps[:],
                            AF.Sigmoid, scale=-1.0, bias=pcol(m, 4 + l))
                for g in range(4):
                    nc.scalar.activation(lnr16[:, g, :], lnr16[:, g, :],
                                         AF.Ln)
                dtu16 = act.tile([128, 4, S], BF16, tag="hn16")
                for g in range(4):
                    nc.vector.scalar_tensor_tensor(
                        dtu16[:, g, :], in0=lnr16[:, g, :], scalar=-1.0,
                        in1=xpa16[:, g, :], op0=OP.mult, op1=OP.mult)

                # ---- selective scan, n in two halves of 8 to bound SBUF
                y16 = act.tile([128, 4, S], BF16, tag="sq")
                ysc = act.tile([128, 4, S], F32, tag="ypart")
                for nh in range(2):
                    B_rep = big.tile([128, 8, S], BF16, tag="B_rep")
                    nc.sync.dma_start(
                        B_rep[:], bc_dram[nh * 8:nh * 8 + 8, :]
                        .unsqueeze(0).broadcast_to([128, 8, S]))
                    C_rep = big.tile([128, 8, S], BF16, tag="C_rep")
                    nc.sync.dma_start(
                        C_rep[:], bc_dram[16 + nh * 8:16 + nh * 8 + 8, :]
                        .unsqueeze(0).broadcast_to([128, 8, S]))
                    for g in range(4):
                        hblk = big.tile([128, 8, S], BF16, tag="hblk")
                        for j in range(8):
                            n = nh * 8 + j
                            dAn = scanp.tile([128, S], BF16, tag="dAn")
                            nc.scalar.activation(
                                dAn[:], lnr16[:, g, :], AF.Exp,
                                scale=pcol(g, 32 + 16 * l + n))
                            dBn = scanp.tile([128, S], BF16, tag="dBn")
                            nc.vector.tensor_tensor(dBn[:], dtu16[:, g, :],
                                                    B_rep[:, j, :], OP.mult)
                            nc.vector.tensor_tensor_scan(
                                hblk[:, j, :], dAn[:], dBn[:], 0.0,
                                OP.mult, OP.add)
                        nc.vector.tensor_tensor(hblk[:], hblk[:], C_rep[:],
                                                OP.mult)
                        nc.vector.tensor_tensor(
                            hblk[:, 0:4, :], hblk[:, 0:4, :],
                            hblk[:, 4:8, :], OP.add)
                        nc.vector.tensor_tensor(
                            hblk[:, 0:2, :], hblk[:, 0:2, :],
                            hblk[:, 2:4, :], OP.add)
                        if nh == 0:
                            nc.vector.tensor_tensor(
                                ysc[:, g, :], hblk[:, 0, :], hblk[:, 1, :],
                                OP.add)
                        else:
                            yg = trans.tile([128, S], F32, tag="yg")
                            nc.vector.tensor_tensor(yg[:], hblk[:, 0, :],
                                                    hblk[:, 1, :], OP.add)
                            nc.vector.tensor_tensor(yg[:], yg[:],
                                                    ysc[:, g, :], OP.add)
                            nc.vector.scalar_tensor_tensor(
                                yg[:], in0=xpa16[:, g, :],
                                scalar=pcol(g, 12 + l),
                                in1=yg[:], op0=OP.mult, op1=OP.add)
                            nc.vector.tensor_tensor(y16[:, g, :], yg[:],
                                                    sz16[:, g, :], OP.mult)

                # ---- out_proj partial + pair allreduce + residual add
                ypart = act.tile([128, 4, S], F32, tag="ypart")
                for m in range(4):
                    for f in range(2):
                        ps = ps_mm.tile([128, 512], F32)
                        for kc in range(4):
                            nc.tensor.matmul(
                                ps[:], outw_sb[:, kc, m * 128:(m + 1) * 128],
                                y16[:, kc, f * 512:(f + 1) * 512],
                                start=(kc == 0), stop=(kc == 3))
                        nc.scalar.activation(
                            ypart[:, m, f * 512:(f + 1) * 512], ps[:],
                            AF.Copy)
                yp_in = dpool.tile([128, 4, S], F32, tag="yp_in")
                yp_out = dpool.tile([128, 4, S], F32, tag="yp_out")
                nc.gpsimd.dma_start(yp_in[:], ypart[:])
                nc.gpsimd.collective_compute(
                    "AllReduce", OP.add, replica_groups=GROUPS,
                    ins=[yp_in[:].opt()], outs=[yp_out[:].opt()])
                ysum = act.tile([128, 4, S], F32, tag="ypart")
                nc.gpsimd.dma_start(ysum[:], yp_out[:])
                for g in range(4):
                    nc.vector.tensor_tensor(h[:, g, :], h[:, g, :],
                                            ysum[:, g, :], OP.add)

        # ---- lin2 + transpose + softmax (all 1024 tokens; host slices)
        with tc.tile_pool(name="tailp", bufs=1) as tp, \
             tc.tile_pool(name="tailt", bufs=2) as tt:
            h16 = tp.tile([128, 4, S], BF16)
            for g in range(4):
                nc.vector.tensor_copy(h16[:, g, :], h[:, g, :])
            l2w = tp.tile([128, 4, 1024], BF16)
            nc.sync.dma_start(l2w[:], dram["lin2w"][:])
            lgt16 = tp.tile([128, 8, S], BF16)
            for m in range(8):
                for f in range(2):
                    ps = ps_mm.tile([128, 512], F32)
                    for kc in range(4):
                        nc.tensor.matmul(
                            ps[:], l2w[:, kc, m * 128:(m + 1) * 128],
                            h16[:, kc, f * 512:(f + 1) * 512],
                            start=(kc == 0), stop=(kc == 3))
                    nc.scalar.activation(lgt16[:, m, f * 512:(f + 1) * 512],
                                         ps[:], AF.Identity,
                                         bias=l2b[:, m:m + 1])
            for tchunk in range(8):
                pst = ps_sm.tile([128, 1024], BF16, tag="ptr")
                for lc in range(8):
                    nc.tensor.transpose(
                        pst[:, lc * 128:(lc + 1) * 128],
                        lgt16[:, lc, tchunk * 128:(tchunk + 1) * 128],
                        ident_sb[:])
                eg = tt.tile([128, 1024], F32, tag="eg")
                nc.scalar.activation(eg[:], pst[:], AF.Exp)
                den = tt.tile([128, 32], F32, tag="den")
                nc.vector.tensor_reduce(
                    den[:], eg[:].rearrange("p (d c) -> p d c", c=32),
                    AX.X, OP.add)
                rec = tt.tile([128, 32], F32, tag="rec")
                nc.vector.reciprocal(rec[:], den[:])
                outt = tt.tile([128, 1024], F32, tag="outt")
                nc.vector.tensor_tensor(
                    outt[:].rearrange("p (d c) -> p d c", c=32),
                    eg[:].rearrange("p (d c) -> p d c", c=32),
                    rec[:].unsqueeze(2).broadcast_to([128, 32, 32]), OP.mult)
                nc.sync.dma_start(out_d[tchunk * 128:(tchunk + 1) * 128, :],
                                  outt[:])


def _build_nc():
    nc = bacc.Bacc("TRN2", target_bir_lowering=False, debug=False,
                   num_devices=8)
    dram = {}
    def din(name, shape, dt=BF16):
        dram[name] = nc.dram_tensor(name, shape, dt, kind="ExternalInput").ap()

    din("xT", [128, 8, S])
    din("lin1w", [128, 8, 512])
    din("lin2w", [128, 4, 1024])
    din("inw", [NLAYERS, 128, 4, 1024])
    din("outw", [NLAYERS, 128, 4, 512])
    din("xprojw", [NLAYERS, 128, 4, 64])
    din("dtw", [NLAYERS, 32, 512])
    din("pvec", [128, 4, NV], F32)
    din("lin1bT", [128, 4], F32)
    din("lin2bT", [128, 8], F32)
    din("ones1", [128, 1])
    din("ident", [128, 128])
    out_d = nc.dram_tensor("out_full", [S, LATENT], F32,
                           kind="ExternalOutput").ap()
    with tile.TileContext(nc) as tc:
        _body(nc, tc, dram, out_d)
    nc.compile()
    return nc


def _prep_inputs(x, lin1_w, lin1_b, norm_w, in_w, conv_w, conv_b, xproj_w,
                 dt_w, dt_b, A_log, Dp, out_w, lin2_w, lin2_b):
    bf = ml_dtypes.bfloat16
    f32 = np.float32
    x = np.asarray(x, f32)
    negA = np.exp(np.asarray(A_log, f32))                 # = n, (L, 1024, 16)
    in_w = np.asarray(in_w, f32)
    shared = {}
    shared["lin1w"] = np.ascontiguousarray(
        np.asarray(lin1_w, f32).reshape(8, 128, 512).transpose(1, 0, 2)
    ).astype(bf)
    shared["lin2w"] = np.ascontiguousarray(
        np.asarray(lin2_w, f32).reshape(4, 128, 1024).transpose(1, 0, 2)
    ).astype(bf)
    shared["lin1bT"] = np.ascontiguousarray(
        np.asarray(lin1_b, f32).reshape(4, 128).T)
    shared["lin2bT"] = np.ascontiguousarray(
        np.asarray(lin2_b, f32).reshape(8, 128).T)
    shared["ones1"] = np.ones((128, 1), bf)
    shared["ident"] = np.eye(128, dtype=f32).astype(bf)

    in_maps = []
    for c in range(8):
        b, half = c // 2, c % 2
        sl = slice(half * D_LOC, (half + 1) * D_LOC)
        m = dict(shared)
        m["xT"] = np.ascontiguousarray(
            x[b].T.reshape(8, 128, S).transpose(1, 0, 2)).astype(bf)
        inw = np.concatenate([in_w[:, :, sl],
                              in_w[:, :, 1024 + half * 512:
                                   1024 + (half + 1) * 512]], axis=2)
        m["inw"] = np.ascontiguousarray(
            inw.reshape(NLAYERS, 4, 128, 1024).transpose(0, 2, 1, 3)
        ).astype(bf)
        m["outw"] = np.ascontiguousarray(
            np.asarray(out_w, f32)[:, sl, :].reshape(NLAYERS, 4, 128, 512)
            .transpose(0, 2, 1, 3)).astype(bf)
        m["xprojw"] = np.ascontiguousarray(
            np.asarray(xproj_w, f32)[:, sl, :].reshape(NLAYERS, 4, 128, 64)
            .transpose(0, 2, 1, 3)).astype(bf)
        m["dtw"] = np.ascontiguousarray(
            np.asarray(dt_w, f32)[:, :, sl]).astype(bf)
        pvec = np.zeros((4, 128, NV), f32)
        for l in range(NLAYERS):
            pvec[:, :, l] = np.asarray(norm_w, f32)[l].reshape(4, 128)
            pvec[:, :, 4 + l] = -np.asarray(dt_b, f32)[l, sl].reshape(4, 128)
            pvec[:, :, 8 + l] = np.asarray(conv_b, f32)[l, sl].reshape(4, 128)
            pvec[:, :, 12 + l] = np.asarray(Dp, f32)[l, sl].reshape(4, 128)
            for k in range(KCONV):
                pvec[:, :, 16 + 4 * l + k] = \
                    np.asarray(conv_w, f32)[l, sl, k].reshape(4, 128)
            for n in range(N):
                pvec[:, :, 32 + 16 * l + n] = negA[l, sl, n].reshape(4, 128)
        m["pvec"] = np.ascontiguousarray(pvec.transpose(1, 0, 2))
        in_maps.append(m)
    return in_maps


def kernel(**inputs) -> np.ndarray:
    if "nc" not in _CACHE:
        _CACHE["nc"] = _build_nc()
    nc = _CACHE["nc"]
    in_maps = _prep_inputs(**inputs)
    res = run_bass_kernel_spmd(nc, in_maps, list(range(8)))
    out = np.zeros((BATCH, S, LATENT), np.float32)
    for b in range(BATCH):
        out[b, 0:512] = res.results[2 * b]["out_full"][0:512]
        out[b, 512:1024] = res.results[2 * b + 1]["out_full"][512:1024]
    return out


# revision 28
# speedup vs baseline: 88.4990x; 88.4990x over previous
"""Trainium2 Bass kernel for the 4-layer Mamba-style GBM model.

Sharding: 8 cores = 4 batches x 2 d_inner halves. Each core handles one
batch and one 512-channel half of d_inner; the two cores of a batch pair
all-reduce the xproj output (dbl) and the out_proj partial sums.

Layout: activations are feature-major in SBUF ([d on partitions, t on
free]).  The selective scan runs as 64 native tensor_tensor_scan calls
per layer (4 d-groups x 16 state dims) with decays dA_n = exp(n*ln r)
produced on the scalar engine from lnr = Ln(sigmoid(-dt_raw)), using
softplus(a) = -ln(sigmoid(-a)).
"""
import sys
sys.path.insert(0, "/opt/trn_rl_repo")

import numpy as np
import ml_dtypes

import concourse.bacc as bacc
import concourse.tile as tile
from concourse import mybir
from concourse.bass_utils import run_bass_kernel_spmd

F32 = mybir.dt.float32
BF16 = mybir.dt.bfloat16
AF = mybir.ActivationFunctionType
OP = mybir.AluOpType
AX = mybir.AxisListType

D_MODEL = 512
D_LOC = 512          # d_inner half per core
N = 16               # d_state
S = 1024
KCONV = 4
NLAYERS = 4
LATENT = 1024
BATCH = 4
GROUPS = [[0, 1], [2, 3], [4, 5], [6, 7]]
NV = 96              # pvec columns

_CACHE = {}
NO_CC = False  # replace collectives with local copies (for TimelineSim)
DBN_GP = True    # odd-j dBu multiplies on GPSIMD
CONV_GP = False  # conv STT chain on GPSIMD
HADD_GP = False  # residual h-add on GPSIMD
DTU_GP = False   # dtu/hn STTs on GPSIMD
GATE_GP = False  # y gate mult on GPSIMD
CM_GP = False    # half the C-multiplies on GPSIMD
MERGED_CC = False  # single full-width collectives instead of t-halved


def _body(nc, tc, dram, out_d):
    import contextlib
    ctx = contextlib.ExitStack()
    with ctx:
        persist = ctx.enter_context(tc.tile_pool(name="persist", bufs=1))
        wbig = ctx.enter_context(tc.tile_pool(name="wbig", bufs=1))
        wsm = ctx.enter_context(tc.tile_pool(name="wsm", bufs=2))
        act = ctx.enter_context(tc.tile_pool(name="act", bufs=1))
        trans = ctx.enter_context(tc.tile_pool(name="trans", bufs=2))
        scanp = ctx.enter_context(tc.tile_pool(name="scanp", bufs=4))
        ps_mm = ctx.enter_context(tc.tile_pool(name="ps_mm", bufs=2, space="PSUM"))
        ps_sm = ctx.enter_context(tc.tile_pool(name="ps_sm", bufs=1, space="PSUM"))
        dpool = ctx.enter_context(tc.tile_pool(name="dpool", bufs=2, space="DRAM"))

        # ---- persistent small tensors
        pv = persist.tile([128, 4, NV], F32)
        nc.sync.dma_start(pv[:], dram["pvec"][:])
        l1b = persist.tile([128, 4], F32)
        nc.sync.dma_start(l1b[:], dram["lin1bT"][:])
        l2b = persist.tile([128, 8], F32)
        nc.sync.dma_start(l2b[:], dram["lin2bT"][:])
        ones_sb = persist.tile([128, 1], BF16)
        nc.sync.dma_start(ones_sb[:], dram["ones1"][:])
        ident_sb = persist.tile([128, 128], BF16)
        nc.sync.dma_start(ident_sb[:], dram["ident"][:])

        def pcol(g, c):
            return pv[:, g, c:c + 1]

        eps_t = persist.tile([1, 1], F32)
        nc.gpsimd.memset(eps_t[:], 1e-5)

        h = persist.tile([128, 4, S], F32)

        # ---- lin1: h = lin1w.T @ xT + b   (scoped pool, freed after)
        with tc.tile_pool(name="lin1p", bufs=1) as lp:
            xT_sb = lp.tile([128, 8, S], BF16)
            nc.sync.dma_start(xT_sb[:], dram["xT"][:])
            l1w = lp.tile([128, 8, 512], BF16)
            nc.sync.dma_start(l1w[:], dram["lin1w"][:])
            for m in range(4):
                for f in range(2):
                    ps = ps_mm.tile([128, 512], F32)
                    for kc in range(8):
                        nc.tensor.matmul(
                            ps[:], l1w[:, kc, m * 128:(m + 1) * 128],
                            xT_sb[:, kc, f * 512:(f + 1) * 512],
                            start=(kc == 0), stop=(kc == 7))
                    nc.scalar.activation(h[:, m, f * 512:(f + 1) * 512],
                                         ps[:], AF.Identity,
                                         bias=l1b[:, m:m + 1])

        # ---- layers (big scan tiles in a scoped pool, freed before tail)
        with tc.tile_pool(name="bigp", bufs=1) as big:
            for l in range(NLAYERS):
                inw_sb = wbig.tile([128, 4, 1024], BF16, tag="inw")
                nc.sync.dma_start(inw_sb[:], dram["inw"][l])
                outw_sb = wbig.tile([128, 4, 512], BF16, tag="outw")
                nc.sync.dma_start(outw_sb[:], dram["outw"][l])
                xprojw_sb = wsm.tile([128, 4, 64], BF16, tag="xprojw")
                nc.sync.dma_start(xprojw_sb[:], dram["xprojw"][l])
                dtw_sb = wsm.tile([32, 512], BF16, tag="dtw")
                nc.sync.dma_start(dtw_sb[:], dram["dtw"][l])

                # rmsnorm -> hn16 (t-halved; f=0 overlaps f=1 allreduce)
                sq = act.tile([128, 4, S], BF16, tag="sq")
                s_t = trans.tile([1, S], F32, tag="s_t")
                s_rep = trans.tile([128, S], F32, tag="s_rep")
                hn16 = act.tile([128, 4, S], BF16, tag="hn16")
                for f in range(2):
                    o = f * 512
                    for g in range(4):
                        nc.scalar.activation(sq[:, g, o:o + 512],
                                             h[:, g, o:o + 512], AF.Square)
                    pss = ps_sm.tile([1, 512], F32, tag="pnorm")
                    for kc in range(4):
                        nc.tensor.matmul(pss[:], ones_sb[:],
                                         sq[:, kc, o:o + 512],
                                         start=(kc == 0), stop=(kc == 3))
                    nc.scalar.activation(s_t[:, o:o + 512], pss[:], AF.Ln,
                                         scale=1.0 / D_MODEL, bias=eps_t[:])
                    nc.scalar.activation(s_t[:, o:o + 512],
                                         s_t[:, o:o + 512], AF.Exp,
                                         scale=-0.5)
                    s_dram = dpool.tile([1, 512], F32, tag="s_dram")
                    nc.sync.dma_start(s_dram[:], s_t[:, o:o + 512])
                    nc.sync.dma_start(
                        s_rep[:, o:o + 512],
                        s_dram[:].broadcast_to([128, 512]))
                    for g in range(4):
                        nc.vector.scalar_tensor_tensor(
                            hn16[:, g, o:o + 512], in0=h[:, g, o:o + 512],
                            scalar=pcol(g, l),
                            in1=s_rep[:, o:o + 512], op0=OP.mult,
                            op1=OP.mult)

                # in_proj -> xp_pad (pre-activation), sz16 = silu(z)
                xp_pad = act.tile([128, 4, S + 3], BF16, tag="xp_pad")
                nc.gpsimd.memset(xp_pad[:, :, 0:3], 0.0)
                sz16 = act.tile([128, 4, S], BF16, tag="sz16")
                for m in range(8):
                    for f in range(2):
                        ps = ps_mm.tile([128, 512], F32)
                        for kc in range(4):
                            nc.tensor.matmul(
                                ps[:], inw_sb[:, kc, m * 128:(m + 1) * 128],
                                hn16[:, kc, f * 512:(f + 1) * 512],
                                start=(kc == 0), stop=(kc == 3))
                        if m < 4:
                            nc.scalar.activation(
                                xp_pad[:, m, 3 + f * 512: 3 + (f + 1) * 512],
                                ps[:], AF.Copy)
                        else:
                            nc.scalar.activation(
                                sz16[:, m - 4, f * 512:(f + 1) * 512],
                                ps[:], AF.Silu)

                # causal depthwise conv + bias + silu -> xpa16
                # (t-halved; conv fh=1 fills DVE during fh=0 allreduce)
                xpa16 = act.tile([128, 4, S], BF16, tag="xpa16")
                cv_eng = nc.gpsimd if CONV_GP else nc.vector
                for fh in range(2):
                    for g in range(4):
                        o = fh * 512
                        c0 = trans.tile([128, 512], BF16, tag="conv",
                                        name=f"cv{fh}_{g}")
                        cv_eng.tensor_scalar_mul(c0[:], xp_pad[:, g, o:o + 512],
                                                 pcol(g, 16 + 4 * l + 0))
                        for k in range(1, KCONV):
                            c1 = trans.tile([128, 512], BF16, tag="conv",
                                            name=f"cv{fh}_{g}_{k}")
                            cv_eng.scalar_tensor_tensor(
                                c1[:], in0=xp_pad[:, g, o + k:o + k + 512],
                                scalar=pcol(g, 16 + 4 * l + k),
                                in1=c0[:], op0=OP.mult, op1=OP.add)
                            c0 = c1
                        nc.scalar.activation(xpa16[:, g, o:o + 512], c0[:],
                                             AF.Silu, bias=pcol(g, 8 + l))

                # xproj -> dbl partial -> pair allreduce in bf16
                # (B/C broadcasts read the collective output directly)
                dbl16 = trans.tile([64, S], BF16, tag="dbl16")
                dbl_outs = []
                dblp_full = trans.tile([64, S], BF16, tag="dblp")
                for fh in range(2):
                    o = fh * 512
                    psx = ps_sm.tile([64, 512], F32, tag="pxproj")
                    for kc in range(4):
                        nc.tensor.matmul(psx[:], xprojw_sb[:, kc, :],
                                         xpa16[:, kc, o:o + 512],
                                         start=(kc == 0), stop=(kc == 3))
                    nc.scalar.activation(dblp_full[:, o:o + 512], psx[:],
                                         AF.Copy)
                    if not MERGED_CC:
                        dbl_in = dpool.tile([64, 512], BF16, tag="dbl_in")
                        dbl_out = dpool.tile([64, 512], BF16, tag="dbl_out")
                        nc.gpsimd.dma_start(dbl_in[:],
                                            dblp_full[:, o:o + 512])
                        if NO_CC:
                            nc.gpsimd.dma_start(dbl_out[:], dbl_in[:])
                        else:
                            nc.gpsimd.collective_compute(
                                "AllReduce", OP.add, replica_groups=GROUPS,
                                ins=[dbl_in[:].opt()],
                                outs=[dbl_out[:].opt()])
                        dbl_outs.append(dbl_out)
                        nc.gpsimd.dma_start(dbl16[:, o:o + 512], dbl_out[:])
                if MERGED_CC:
                    dbl_in = dpool.tile([64, S], BF16, tag="dbl_in")
                    dbl_out = dpool.tile([64, S], BF16, tag="dbl_out")
                    nc.gpsimd.dma_start(dbl_in[:], dblp_full[:])
                    if NO_CC:
                        nc.gpsimd.dma_start(dbl_out[:], dbl_in[:])
                    else:
                        nc.gpsimd.collective_compute(
                            "AllReduce", OP.add, replica_groups=GROUPS,
                            ins=[dbl_in[:].opt()], outs=[dbl_out[:].opt()])
                    dbl_outs = [dbl_out, dbl_out]
                    nc.gpsimd.dma_start(dbl16[:], dbl_out[:])

                # dt-proj -> lnr = Ln(sigmoid(-(dt_raw + dt_b))) = -dt
                lnr16 = act.tile([128, 4, S], BF16, tag="xp_pad")
                for m in range(4):
                    for f in range(2):
                        ps = ps_mm.tile([128, 512], F32)
                        nc.tensor.matmul(
                            ps[:], dtw_sb[:, m * 128:(m + 1) * 128],
                            dbl16[0:32, f * 512:(f + 1) * 512],
                            start=True, stop=True)
                        nc.scalar.activation(
                            lnr16[:, m, f * 512:(f + 1) * 512], ps[:],
                            AF.Sigmoid, scale=-1.0, bias=pcol(m, 4 + l))
                for fh in range(2):
                    for g in range(4):
                        o = fh * 512
                        nc.scalar.activation(lnr16[:, g, o:o + 512],
                                             lnr16[:, g, o:o + 512], AF.Ln)
                dtu16 = act.tile([128, 4, S], BF16, tag="hn16")
                dtu_eng = nc.gpsimd if DTU_GP else nc.vector
                for fh in range(2):
                    for g in range(4):
                        o = fh * 512
                        dtu_eng.scalar_tensor_tensor(
                            dtu16[:, g, o:o + 512],
                            in0=lnr16[:, g, o:o + 512], scalar=-1.0,
                            in1=xpa16[:, g, o:o + 512],
                            op0=OP.mult, op1=OP.mult)

                # ---- selective scan, n in two halves of 8 to bound SBUF
                y16 = act.tile([128, 4, S], BF16, tag="sq")
                ysc = act.tile([128, 4, S], BF16, tag="ysc")
                for nh in range(2):
                    B_rep = big.tile([128, 8, S], BF16, tag="B_rep")
                    C_rep = big.tile([128, 8, S], BF16, tag="C_rep")
                    for fh in range(2):
                        o = fh * 512
                        oo = o if MERGED_CC else 0
                        nc.sync.dma_start(
                            B_rep[:, :, o:o + 512],
                            dbl_outs[fh][32 + nh * 8:32 + nh * 8 + 8,
                                         oo:oo + 512]
                            .unsqueeze(0).broadcast_to([128, 8, 512]))
                        nc.sync.dma_start(
                            C_rep[:, :, o:o + 512],
                            dbl_outs[fh][48 + nh * 8:48 + nh * 8 + 8,
                                         oo:oo + 512]
                            .unsqueeze(0).broadcast_to([128, 8, 512]))
                    for g in range(4):
                        hblk = big.tile([128, 8, S], BF16, tag="hblk")
                        for j in range(8):
                            n = nh * 8 + j
                            dAn = scanp.tile([128, S], BF16, tag="dAn")
                            nc.scalar.activation(
                                dAn[:], lnr16[:, g, :], AF.Exp,
                                scale=pcol(g, 32 + 16 * l + n))
                            dBn = scanp.tile([128, S], BF16, tag="dBn")
                            dbn_eng = nc.gpsimd if (DBN_GP and j % 2 == 1) else nc.vector
                            dbn_eng.tensor_tensor(dBn[:], dtu16[:, g, :],
                                                  B_rep[:, j, :], OP.mult)
                            nc.vector.tensor_tensor_scan(
                                hblk[:, j, :], dAn[:], dBn[:], 0.0,
                                OP.mult, OP.add)
                        cm_eng = nc.gpsimd if (CM_GP and g % 2 == 0) else nc.vector
                        cm_eng.tensor_tensor(hblk[:], hblk[:], C_rep[:],
                                             OP.mult)
                        nc.vector.tensor_tensor(
                            hblk[:, 0:4, :], hblk[:, 0:4, :],
                            hblk[:, 4:8, :], OP.add)
                        nc.vector.tensor_tensor(
                            hblk[:, 0:2, :], hblk[:, 0:2, :],
                            hblk[:, 2:4, :], OP.add)
                        if nh == 0:
                            nc.vector.tensor_tensor(
                                ysc[:, g, :], hblk[:, 0, :], hblk[:, 1, :],
                                OP.add)
                        else:
                            yg = trans.tile([128, S], BF16, tag="yg")
                            nc.vector.tensor_tensor(yg[:], hblk[:, 0, :],
                                                    hblk[:, 1, :], OP.add)
                            nc.vector.tensor_tensor(yg[:], yg[:],
                                                    ysc[:, g, :], OP.add)
                            nc.vector.scalar_tensor_tensor(
                                yg[:], in0=xpa16[:, g, :],
                                scalar=pcol(g, 12 + l),
                                in1=yg[:], op0=OP.mult, op1=OP.add)
                            gt_eng = nc.gpsimd if GATE_GP else nc.vector
                            gt_eng.tensor_tensor(y16[:, g, :], yg[:],
                                                 sz16[:, g, :], OP.mult)

                # ---- out_proj partial + pair allreduce + residual add
                ypart = act.tile([128, 4, S], BF16, tag="ypart")
                ysum = act.tile([128, 4, S], BF16, tag="ysum")
                ha_eng = nc.gpsimd if HADD_GP else nc.vector
                with tc.tile_pool(name="ps_out", bufs=1,
                                  space="PSUM") as ps_out:
                    for f in range(2):
                        pss = [ps_out.tile([128, 512], F32, tag=f"po{m}",
                                           name=f"po{f}_{m}")
                               for m in range(4)]
                        for kc in range(4):
                            for m in range(4):
                                nc.tensor.matmul(
                                    pss[m][:],
                                    outw_sb[:, kc, m * 128:(m + 1) * 128],
                                    y16[:, kc, f * 512:(f + 1) * 512],
                                    start=(kc == 0), stop=(kc == 3))
                        for m in range(4):
                            nc.scalar.activation(
                                ypart[:, m, f * 512:(f + 1) * 512],
                                pss[m][:], AF.Copy)
                        o = f * 512
                        if not MERGED_CC:
                            yp_in = dpool.tile([128, 4, 512], BF16,
                                               tag="yp_in")
                            yp_out = dpool.tile([128, 4, 512], BF16,
                                                tag="yp_out")
                            nc.gpsimd.dma_start(yp_in[:],
                                                ypart[:, :, o:o + 512])
                            if NO_CC:
                                nc.gpsimd.dma_start(yp_out[:], yp_in[:])
                            else:
                                nc.gpsimd.collective_compute(
                                    "AllReduce", OP.add,
                                    replica_groups=GROUPS,
                                    ins=[yp_in[:].opt()],
                                    outs=[yp_out[:].opt()])
                            nc.gpsimd.dma_start(ysum[:, :, o:o + 512],
                                                yp_out[:])
                            for g in range(4):
                                ha_eng.tensor_tensor(
                                    h[:, g, o:o + 512], h[:, g, o:o + 512],
                                    ysum[:, g, o:o + 512], OP.add)
                    if MERGED_CC:
                        yp_in = dpool.tile([128, 4, S], BF16, tag="yp_in")
                        yp_out = dpool.tile([128, 4, S], BF16, tag="yp_out")
                        nc.gpsimd.dma_start(yp_in[:], ypart[:])
                        if NO_CC:
                            nc.gpsimd.dma_start(yp_out[:], yp_in[:])
                        else:
                            nc.gpsimd.collective_compute(
                                "AllReduce", OP.add, replica_groups=GROUPS,
                                ins=[yp_in[:].opt()], outs=[yp_out[:].opt()])
                        nc.gpsimd.dma_start(ysum[:], yp_out[:])
                        for g in range(4):
                            ha_eng.tensor_tensor(h[:, g, :], h[:, g, :],
                                                 ysum[:, g, :], OP.add)

        # ---- lin2 + transpose + softmax (all 1024 tokens; host slices)
        with tc.tile_pool(name="tailp", bufs=1) as tp, \
             tc.tile_pool(name="tailt", bufs=2) as tt:
            h16 = tp.tile([128, 4, S], BF16)
            for g in range(4):
                nc.vector.tensor_copy(h16[:, g, :], h[:, g, :])
            l2w = tp.tile([128, 4, 1024], BF16)
            nc.sync.dma_start(l2w[:], dram["lin2w"][:])
            lgt16 = tp.tile([128, 8, S], BF16)
            ps_tail = ctx.enter_context(
                tc.tile_pool(name="ps_tail", bufs=1, space="PSUM"))
            for f in range(2):
                for m in range(8):
                    ps = ps_mm.tile([128, 512], F32)
                    for kc in range(4):
                        nc.tensor.matmul(
                            ps[:], l2w[:, kc, m * 128:(m + 1) * 128],
                            h16[:, kc, f * 512:(f + 1) * 512],
                            start=(kc == 0), stop=(kc == 3))
                    nc.scalar.activation(lgt16[:, m, f * 512:(f + 1) * 512],
                                         ps[:], AF.Identity,
                                         bias=l2b[:, m:m + 1])
            for tchunk in range(8):
                pst = ps_tail.tile([128, 1024], BF16, tag="ptr")
                for lc in range(8):
                    nc.tensor.transpose(
                        pst[:, lc * 128:(lc + 1) * 128],
                        lgt16[:, lc, tchunk * 128:(tchunk + 1) * 128],
                        ident_sb[:])
                eg = tt.tile([128, 1024], F32, tag="eg")
                nc.scalar.activation(eg[:], pst[:], AF.Exp)
                den = tt.tile([128, 32], F32, tag="den")
                nc.vector.tensor_reduce(
                    den[:], eg[:].rearrange("p (d c) -> p d c", c=32),
                    AX.X, OP.add)
                rec = tt.tile([128, 32], F32, tag="rec")
                nc.vector.reciprocal(rec[:], den[:])
                outt = tt.tile([128, 1024], F32, tag="outt")
                nc.vector.tensor_tensor(
                    outt[:].rearrange("p (d c) -> p d c", c=32),
                    eg[:].rearrange("p (d c) -> p d c", c=32),
                    rec[:].unsqueeze(2).broadcast_to([128, 32, 32]), OP.mult)
                nc.sync.dma_start(out_d[tchunk * 128:(tchunk + 1) * 128, :],
                                  outt[:])


def _build_nc():
    nc = bacc.Bacc("TRN2", target_bir_lowering=False, debug=False,
                   num_devices=8)
    dram = {}
    def din(name, shape, dt=BF16):
        dram[name] = nc.dram_tensor(name, shape, dt, kind="ExternalInput").ap()

    din("xT", [128, 8, S])
    din("lin1w", [128, 8, 512])
    din("lin2w", [128, 4, 1024])
    din("inw", [NLAYERS, 128, 4, 1024])
    din("outw", [NLAYERS, 128, 4, 512])
    din("xprojw", [NLAYERS, 128, 4, 64])
    din("dtw", [NLAYERS, 32, 512])
    din("pvec", [128, 4, NV], F32)
    din("lin1bT", [128, 4], F32)
    din("lin2bT", [128, 8], F32)
    din("ones1", [128, 1])
    din("ident", [128, 128])
    out_d = nc.dram_tensor("out_full", [S, LATENT], F32,
                           kind="ExternalOutput").ap()
    with tile.TileContext(nc) as tc:
        _body(nc, tc, dram, out_d)
    nc.compile()
    return nc


def _prep_inputs(x, lin1_w, lin1_b, norm_w, in_w, conv_w, conv_b, xproj_w,
                 dt_w, dt_b, A_log, Dp, out_w, lin2_w, lin2_b):
    bf = ml_dtypes.bfloat16
    f32 = np.float32
    x = np.asarray(x, f32)
    negA = np.exp(np.asarray(A_log, f32))                 # = n, (L, 1024, 16)
    in_w = np.asarray(in_w, f32)
    shared = {}
    shared["lin1w"] = np.ascontiguousarray(
        np.asarray(lin1_w, f32).reshape(8, 128, 512).transpose(1, 0, 2)
    ).astype(bf)
    shared["lin2w"] = np.ascontiguousarray(
        np.asarray(lin2_w, f32).reshape(4, 128, 1024).transpose(1, 0, 2)
    ).astype(bf)
    shared["lin1bT"] = np.ascontiguousarray(
        np.asarray(lin1_b, f32).reshape(4, 128).T)
    shared["lin2bT"] = np.ascontiguousarray(
        np.asarray(lin2_b, f32).reshape(8, 128).T)
    shared["ones1"] = np.ones((128, 1), bf)
    shared["ident"] = np.eye(128, dtype=f32).astype(bf)

    in_maps = []
    for c in range(8):
        b, half = c // 2, c % 2
        sl = slice(half * D_LOC, (half + 1) * D_LOC)
        m = dict(shared)
        m["xT"] = np.ascontiguousarray(
            x[b].T.reshape(8, 128, S).transpose(1, 0, 2)).astype(bf)
        inw = np.concatenate([in_w[:, :, sl],
                              in_w[:, :, 1024 + half * 512:
                                   1024 + (half + 1) * 512]], axis=2)
        m["inw"] = np.ascontiguousarray(
            inw.reshape(NLAYERS, 4, 128, 1024).transpose(0, 2, 1, 3)
        ).astype(bf)
        m["outw"] = np.ascontiguousarray(
            np.asarray(out_w, f32)[:, sl, :].reshape(NLAYERS, 4, 128, 512)
            .transpose(0, 2, 1, 3)).astype(bf)
        m["xprojw"] = np.ascontiguousarray(
            np.asarray(xproj_w, f32)[:, sl, :].reshape(NLAYERS, 4, 128, 64)
            .transpose(0, 2, 1, 3)).astype(bf)
        m["dtw"] = np.ascontiguousarray(
            np.asarray(dt_w, f32)[:, :, sl]).astype(bf)
        pvec = np.zeros((4, 128, NV), f32)
        for l in range(NLAYERS):
            pvec[:, :, l] = np.asarray(norm_w, f32)[l].reshape(4, 128)
            pvec[:, :, 4 + l] = -np.asarray(dt_b, f32)[l, sl].reshape(4, 128)
            pvec[:, :, 8 + l] = np.asarray(conv_b, f32)[l, sl].reshape(4, 128)
            pvec[:, :, 12 + l] = np.asarray(Dp, f32)[l, sl].reshape(4, 128)
            for k in range(KCONV):
                pvec[:, :, 16 + 4 * l + k] = \
                    np.asarray(conv_w, f32)[l, sl, k].reshape(4, 128)
            for n in range(N):
                pvec[:, :, 32 + 16 * l + n] = negA[l, sl, n].reshape(4, 128)
        m["pvec"] = np.ascontiguousarray(pvec.transpose(1, 0, 2))
        in_maps.append(m)
    return in_maps


def kernel(**inputs) -> np.ndarray:
    if "nc" not in _CACHE:
        _CACHE["nc"] = _build_nc()
    nc = _CACHE["nc"]
    in_maps = _prep_inputs(**inputs)
    res = run_bass_kernel_spmd(nc, in_maps, list(range(8)))
    out = np.zeros((BATCH, S, LATENT), np.float32)
    for b in range(BATCH):
        out[b, 0:512] = res.results[2 * b]["out_full"][0:512]
        out[b, 512:1024] = res.results[2 * b + 1]["out_full"][512:1024]
    return out
